# revision 6
# baseline (speedup 1.0000x reference)
"""Trainium2 Bass kernel for nn_HGNER (windowed bi-LSTM + attention + linear head).

Sharding: 8 cores x 128 tokens (data-parallel over the flattened (B,L) token
axis; each core gets half of one batch row plus a 4-token halo). Small LSTM /
linear params are replicated to every core.

Structure vs the previous version:
 - G injection is fused into the mandatory 3rd W_hh contraction chunk: the
   second DoubleRow pass contracts (id8, whh_k2) x (G_j_slice, h_k2) via
   custom-stride access patterns, so the 3 standalone identity-matmul injects
   per step are gone.
 - Per-step PSUM is split into a 3-bank ifo tile and a 1-bank g tile (2 ring
   slots each = 8 banks); gates for the g-gate are computed first so tanh(g)
   overlaps the remaining matmuls.
 - All 8 chains are software-pipelined: a round-robin scheduler keeps up to 4
   chains' steps in flight, with later chains' G matmuls and weight DMA
   injected between steps (two G groups per tick while ramping up).
 - Host pre-transposes every DMA'd tensor into its SBUF layout (contiguous
   per-partition rows), collapsing thousands of strided DMA descriptors.
 - Softmax uses e^x = (1+tanh(x/2))/(1-tanh(x/2)) so the epilogue reuses the
   already-loaded tanh table instead of reloading the exp table set.
 - Predicate/snapshot buffers use stride-5 slot layouts so their access
   patterns stay 3-D (keeps CoreSim's view collapsing consistent with the
   strided outputs they pair with).

Note: the weight-DMA emission order and the contiguous score-matmul block are
load-bearing — reordering the prologue DMAs or interleaving the score matmul
accumulation group with step matmuls makes the NEFF fail at runtime, so leave
those as-is.
"""

import numpy as np
import ml_dtypes
from collections import deque

import concourse.bass as bass
import concourse.bacc as bacc_mod
import concourse.mybir as mybir
from concourse.tile import TileContext
from concourse.bass_types import AP
from concourse.bass_utils import run_bass_kernel_spmd

F32 = mybir.dt.float32
BF16 = mybir.dt.bfloat16
FP8 = mybir.dt.float8e4
U8 = mybir.dt.uint8
AF = mybir.ActivationFunctionType
ALU = mybir.AluOpType
AX = mybir.AxisListType
DR = mybir.MatmulPerfMode.DoubleRow

B, L, D, H, NW, NL = 4, 256, 768, 384, 4, 9
WINDOWS = (3, 5, 7, 9)
NCORES = 8
TPC = 128          # tokens per core
HALO = 4           # max half-window
TH = TPC + 2 * HALO  # 136 tokens incl. halo
DC = D // 128      # 6 chunks of input features
HC = H // 128      # 3 chunks of hidden features
GC = 4 * H // 128  # 12 chunks of gate features
G4 = 4 * H         # 1536
NCH = 2 * NW       # 8 (window, direction) chains
GLEN = GC * TH     # G region length in the per-chain state tile
HOFF = GLEN        # h region offset inside the g tile
GTOT = GLEN + HC * TPC
SCALE = 1.0 / np.sqrt(np.float32(D))
WS = 16.0          # host-side weight scale (fp8 subnormal avoidance)
IWS = 1.0 / WS
EDGE = -224.0      # fp8-storable; /16 => sigma(-14)~8e-7, tanh(-14)~-1

_CACHE = {}


def _pair_ap(base, off_a, off_b, ncols):
    """[128, 2, ncols] AP over one tile: chunk 0 at off_a, chunk 1 at off_b."""
    pstride, psize = base.ap[0]
    return AP(base.tensor, base.offset + off_a,
              [(pstride, psize), (off_b - off_a, 2), (1, ncols)])


def _build():
    nc = bacc_mod.Bacc()

    # ---- DRAM I/O (all pre-transposed to SBUF layout on host) ----
    xt_d = nc.dram_tensor("xt", [128, DC * TH], FP8, kind="ExternalInput")
    xb_d = nc.dram_tensor("xb", [128, DC * TPC], BF16, kind="ExternalInput")
    wih_d = nc.dram_tensor("wih", [NCH, 128, DC * G4], FP8,
                           kind="ExternalInput")
    whh_d = nc.dram_tensor("whh", [NCH, 128, 2 * G4], FP8,
                           kind="ExternalInput")
    # per chain: [id8 | whh_k2 gate blocks 0..11] for the fused 2nd DR pass
    w2x_d = nc.dram_tensor("w2x", [NCH, 128, (GC + 1) * 128], FP8,
                           kind="ExternalInput")
    bias_d = nc.dram_tensor("bias", [128, NCH * GC + 30], F32,
                           kind="ExternalInput")
    linw_d = nc.dram_tensor("linw", [128, DC * NL], BF16, kind="ExternalInput")
    linb_d = nc.dram_tensor("linb", [1, NL], BF16, kind="ExternalInput")
    idb_d = nc.dram_tensor("idb", [128, 128], BF16, kind="ExternalInput")
    ones_d = nc.dram_tensor("ones", [128, 1], BF16, kind="ExternalInput")
    onesr_d = nc.dram_tensor("onesr", [1, 512], BF16, kind="ExternalInput")
    # edge predication masks (per-core data; program is identical on all cores)
    pmf_d = nc.dram_tensor("pmf", [128, NW * HC * 5], U8, kind="ExternalInput")
    pmb_d = nc.dram_tensor("pmb", [128, NW * HC * 5], U8, kind="ExternalInput")
    out_d = nc.dram_tensor("out", [NL, TPC], F32, kind="ExternalOutput")

    with TileContext(nc) as tc:
        with (
            tc.tile_pool(name="const", bufs=1) as cpool,
            tc.tile_pool(name="wih", bufs=3) as wih_pool,
            tc.tile_pool(name="whh", bufs=NCH) as whh_pool,
            tc.tile_pool(name="g", bufs=8) as g_pool,
            tc.tile_pool(name="muti", bufs=NCH + 2) as muti_pool,
            tc.tile_pool(name="st", bufs=10) as st_pool,
            tc.tile_pool(name="snap", bufs=NCH) as snap_pool,
            tc.tile_pool(name="tmp", bufs=6) as tmp_pool,
            tc.tile_pool(name="fin", bufs=2) as fin_pool,
            tc.tile_pool(name="pifo", bufs=2, space="PSUM") as pifo_pool,
            tc.tile_pool(name="ptg", bufs=2, space="PSUM") as ptg_pool,
        ):
            weights = {}

            def load_chain_weights(c):
                wihT = wih_pool.tile([128, DC * G4], FP8, tag="wih",
                                     name=f"wih{c}")
                # three piece-DMAs on distinct queues: G pass kp only waits
                # for its own contraction-pair piece
                for kp in range(3):
                    nc.sync.dma_start(
                        out=wihT[:, kp * 2 * G4:(kp + 1) * 2 * G4],
                        in_=wih_d[c][:, kp * 2 * G4:(kp + 1) * 2 * G4])
                whhT = whh_pool.tile([128, 2 * G4], FP8, tag="whh",
                                     name=f"whh{c}")
                nc.sync.dma_start(out=whhT[:], in_=whh_d[c])
                w2xT = whh_pool.tile([128, (GC + 1) * 128], FP8, tag="w2x",
                                     name=f"w2x{c}")
                nc.sync.dma_start(out=w2xT[:], in_=w2x_d[c])
                return (wihT, whhT, w2xT)

            # ---- critical-path DMAs first: x, first chains' weights ----
            xt = cpool.tile([128, DC * TH], FP8, tag="xt")
            nc.sync.dma_start(out=xt[:], in_=xt_d[:])
            biasx = cpool.tile([128, NCH * GC + 30], F32, tag="bias")
            nc.sync.dma_start(out=biasx[:], in_=bias_d[:])
            biasr = biasx[:, 0:NCH * GC]
            pgl = biasx[:, NCH * GC:NCH * GC + 15].bitcast(U8)
            pgr = biasx[:, NCH * GC + 15:NCH * GC + 30].bitcast(U8)
            weights = {}
            for c in (6, 7, 4, 5):
                weights[c] = load_chain_weights(c)
            xb = cpool.tile([128, DC * TPC], BF16, tag="xb")
            nc.sync.dma_start(out=xb[:], in_=xb_d[:])
            ones = cpool.tile([128, 1], BF16, tag="ones")
            nc.sync.dma_start(out=ones[:], in_=ones_d[:])
            pmf = cpool.tile([128, NW * HC * 5], U8, tag="pmf")
            nc.sync.dma_start(out=pmf[:], in_=pmf_d[:])
            pmb = cpool.tile([128, NW * HC * 5], U8, tag="pmb")
            nc.sync.dma_start(out=pmb[:], in_=pmb_d[:])
            negt = cpool.tile([128, GC * 5], FP8, tag="negt")
            nc.vector.memset(negt[:], EDGE)

            # one-time engine touches of DMA-loaded consts: collapse later
            # waits to a single semaphore (instr structs have 1 wait slot)
            wu8 = cpool.tile([128, 1], U8, tag="wu8")
            nc.vector.tensor_copy(wu8[:], pgl[:, 0:1])
            nc.vector.tensor_copy(wu8[:], pgr[:, 0:1])
            nc.vector.tensor_copy(wu8[:], pmf[:, 0:1])
            nc.vector.tensor_copy(wu8[:], pmb[:, 0:1])
            wb0 = cpool.tile([128, 1], BF16, tag="wb0")
            nc.vector.tensor_copy(wb0[:], xb[:, 0:1])
            wb1 = cpool.tile([128, 1], BF16, tag="wb1")
            nc.gpsimd.tensor_copy(wb1[:], xb[:, 0:1])

            # ---- non-critical consts (epilogue) ----
            linw = cpool.tile([128, DC * NL], BF16, tag="linw")
            nc.sync.dma_start(out=linw[:], in_=linw_d[:])
            linb = cpool.tile([1, NL], BF16, tag="linb")
            nc.sync.dma_start(out=linb[:], in_=linb_d[:])
            idb = cpool.tile([128, 128], BF16, tag="idb")
            nc.sync.dma_start(out=idb[:], in_=idb_d[:])
            onesr = cpool.tile([1, 512], BF16, tag="onesr")
            nc.sync.dma_start(out=onesr[:], in_=onesr_d[:])

            xt3 = xt[:].rearrange("p (k t) -> p k t", t=TH)
            xb3 = xb[:].rearrange("p (k t) -> p k t", t=TPC)

            # ---- per-chain state ----
            gtiles, csts, muts, snaps, tsteps = {}, {}, {}, {}, {}
            prods = {}

            def emit_g_group(c, grp):
                wihT = weights[c][0]
                wih3 = wihT[:].rearrange("p (k n) -> p k n", k=DC)
                g = gtiles[c]
                gg = pifo_pool.tile([128, 3 * TH], F32, tag="tifo",
                                    name=f"gg{c}_{grp}")
                for cc in range(3):
                    j = grp * 3 + cc
                    for kp in range(3):
                        nc.tensor.matmul(
                            gg[:, cc * TH:(cc + 1) * TH],
                            lhsT=wih3[:, 2 * kp:2 * kp + 2,
                                      j * 128:(j + 1) * 128],
                            rhs=xt3[:, 2 * kp:2 * kp + 2, :],
                            start=(kp == 0), stop=(kp == 2), perf_mode=DR,
                        )
                for cc in range(3):
                    j = grp * 3 + cc
                    nc.vector.tensor_scalar(
                        g[:, j * TH:(j + 1) * TH],
                        gg[:, cc * TH:(cc + 1) * TH],
                        biasr[:, c * GC + j:c * GC + j + 1], None, ALU.add)

            def emit_edges(c):
                # invalid halo token columns -> EDGE (gates saturate)
                g = gtiles[c]
                g3 = g[:, 0:GLEN].rearrange("p (j t) -> p j t", t=TH)
                nc.vector.copy_predicated(
                    g3[:, :, 0:4],
                    pgl.rearrange("p (j e) -> p j e", e=5)[:, :, 0:4],
                    negt[:].rearrange("p (j e) -> p j e", e=5)[:, :, 0:4],
                )
                nc.vector.copy_predicated(
                    g3[:, :, TH - 4:TH],
                    pgr.rearrange("p (j e) -> p j e", e=5)[:, :, 0:4],
                    negt[:].rearrange("p (j e) -> p j e", e=5)[:, :, 0:4],
                )

            def g_emitter(c):
                g = g_pool.tile([128, GTOT], FP8, tag="g", name=f"g{c}")
                gtiles[c] = g
                sn = snap_pool.tile([128, HC * 5], BF16, tag="snap",
                                    name=f"snap{c}")
                nc.vector.memset(sn[:], 0.0)
                snaps[c] = sn
                mut = muti_pool.tile([128, HC * TPC], BF16, tag="muti",
                                     name=f"muti{c}")
                muts[c] = mut
                for grp in range(4):
                    emit_g_group(c, grp)
                    yield
                emit_edges(c)

            def emit_step(c, t):
                wi, d = c // 2, c % 2
                w = WINDOWS[wi]
                half = w // 2
                o = (t - half) if d == 0 else (half - t)
                lo = HALO + o
                g = gtiles[c]
                g3 = g[:, 0:GLEN].rearrange("p (j t) -> p j t", t=TH)
                if t == 0:
                    src_g = g3[:, 9:12, lo:lo + TPC]
                    src_ifo = g3[:, 0:9, lo:lo + TPC]
                else:
                    whhT, w2xT = weights[c][1], weights[c][2]
                    whh3 = whhT[:].rearrange("p (k n) -> p k n", k=2)
                    h3 = g[:, HOFF:GTOT].rearrange("p (k t) -> p k t", k=HC)
                    tifo = pifo_pool.tile([128, 9 * TPC], F32, tag="tifo",
                                          name=f"tifo{c}_{t}")
                    tgp = ptg_pool.tile([128, 3 * TPC], F32, tag="tgps",
                                        name=f"tgp{c}_{t}")
                    # g-gate chunks first (unblocks tanh_g), then i,f, then o
                    for j in (9, 10, 11, 0, 1, 2, 3, 4, 5, 6, 7, 8):
                        out = (tgp[:, (j - 9) * TPC:(j - 8) * TPC] if j >= 9
                               else tifo[:, j * TPC:(j + 1) * TPC])
                        nc.tensor.matmul(
                            out,
                            lhsT=whh3[:, 0:2, j * 128:(j + 1) * 128],
                            rhs=h3[:, 0:2, :],
                            start=True, stop=False, perf_mode=DR,
                        )
                        # fused pass 2: (id8, whh_k2) x (G_j slice, h_k2)
                        nc.tensor.matmul(
                            out,
                            lhsT=_pair_ap(w2xT, 0, (1 + j) * 128, 128),
                            rhs=_pair_ap(g, j * TH + lo, HOFF + 2 * TPC, TPC),
                            start=False, stop=True, perf_mode=DR,
                        )
                    src_g = tgp[:].rearrange("p (k t) -> p k t", t=TPC)
                    src_ifo = tifo[:].rearrange("p (k t) -> p k t", t=TPC)
                # activations: tanh(g) first, then sigmoid(i,f,o)
                tg = tmp_pool.tile([128, H], BF16, tag="tg", name=f"tg{c}_{t}")
                nc.scalar.activation(
                    tg[:].rearrange("p (c t) -> p c t", t=TPC),
                    src_g, AF.Tanh, scale=IWS)
                sfo = tmp_pool.tile([128, 3 * H], BF16, tag="sfo",
                                    name=f"sfo{c}_{t}")
                nc.scalar.activation(
                    sfo[:].rearrange("p (c t) -> p c t", t=TPC),
                    src_ifo, AF.Sigmoid, scale=IWS)
                # c_new = sig(f)*c + sig(i)*tanh(g)
                cn = st_pool.tile([128, H], BF16, tag="cn", name=f"c{c}_{t}")
                if t > 0:
                    ig = tmp_pool.tile([128, H], BF16, tag="ig",
                                       name=f"ig{c}_{t}")
                    nc.vector.tensor_tensor(ig[:], sfo[:, 0:H], tg[:],
                                            ALU.mult)
                    fc = tmp_pool.tile([128, H], BF16, tag="fc",
                                       name=f"fc{c}_{t}")
                    nc.vector.tensor_tensor(fc[:], sfo[:, H:2 * H],
                                            csts[c][:], ALU.mult)
                    nc.vector.tensor_tensor(cn[:], ig[:], fc[:], ALU.add)
                else:
                    nc.vector.tensor_tensor(cn[:], sfo[:, 0:H], tg[:],
                                            ALU.mult)
                csts[c] = cn
                tcn = tmp_pool.tile([128, H], BF16, tag="tcn",
                                    name=f"tcn{c}_{t}")
                nc.scalar.activation(tcn[:], cn[:], AF.Tanh)
                # h = sig(o) * tanh(c); last step lands in bf16 muti
                if t == w - 1:
                    nc.vector.tensor_tensor(
                        muts[c][:], sfo[:, 2 * H:3 * H], tcn[:], ALU.mult)
                else:
                    nc.vector.tensor_tensor(
                        g[:, HOFF:GTOT], sfo[:, 2 * H:3 * H], tcn[:],
                        ALU.mult)
                    # trailing-edge snapshot: one token column whose last
                    # valid step is t (real only on edge cores; merged
                    # predicated at chain end)
                    if half <= t:
                        if d == 0:
                            tok = TPC - 1 - (t - half)
                            slot = tok - (TPC - 4)
                        else:
                            tok = t - half
                            slot = tok
                        h3n = g[:, HOFF:GTOT].rearrange(
                            "p (k t) -> p k t", k=HC)
                        sn3 = snaps[c][:].rearrange("p (k s) -> p k s", s=5)
                        nc.vector.tensor_copy(
                            sn3[:, :, slot:slot + 1],
                            h3n[:, :, tok:tok + 1])

            nfin = [0]
            prsum = {}

            def emit_finish(c):
                wi, d = c // 2, c % 2
                mut = muts[c]
                m3 = mut[:].rearrange("p (k t) -> p k t", k=HC)
                pm = (pmf if d == 0 else pmb)[:].rearrange(
                    "p (w k s) -> p w k s", w=NW, s=5)
                sn3 = snaps[c][:].rearrange("p (k s) -> p k s", s=5)
                cols = m3[:, :, TPC - 4:TPC] if d == 0 else m3[:, :, 0:4]
                nc.vector.copy_predicated(cols, pm[:, wi, :, 0:4],
                                          sn3[:, :, 0:4])
                # attention score product + partial score matmuls: both
                # hide under the remaining chains' steps
                pr = tmp_pool.tile([128, HC * TPC], BF16, tag="pr",
                                   name=f"pr{c}", bufs=NCH)
                nc.gpsimd.tensor_tensor(
                    pr[:], mut[:], xb3[:, d * HC:(d + 1) * HC, :], ALU.mult)
                prods[c] = pr
                if d == 1:
                    pr2 = tmp_pool.tile([128, HC * TPC], BF16, tag="pr2",
                                        name=f"pr2_{wi}", bufs=NW)
                    nc.gpsimd.tensor_tensor(pr2[:], prods[wi * 2][:], pr[:],
                                            ALU.add)
                    prsum[wi] = pr2
                nfin[0] += 1

            # ---- schedule: round-robin over chains, G groups injected ----
            order = [6, 7, 4, 5, 2, 3, 0, 1]
            gq = deque(order)
            ready = deque()
            cur_g = None
            dma_next = 4  # chains 2,3,0,1 DMA'd lazily
            # bootstrap: G + t0 for the first two chains
            for _ in range(2):
                c = gq.popleft()
                for _ in g_emitter(c):
                    pass
                emit_step(c, 0)
                tsteps[c] = 1
                ready.append(c)
            while ready or gq or cur_g is not None:
                if ready:
                    c = ready.popleft()
                    emit_step(c, tsteps[c])
                    tsteps[c] += 1
                    if tsteps[c] == WINDOWS[c // 2]:
                        emit_finish(c)
                    else:
                        ready.append(c)
                # inject G groups per tick (denser while few chains active)
                for _ in range(2 if len(ready) < 4 else 1):
                    if cur_g is None and gq:
                        cg = gq.popleft()
                        if dma_next < len(order):
                            weights[order[dma_next]] = load_chain_weights(
                                order[dma_next])
                            dma_next += 1
                        cur_g = (cg, g_emitter(cg))
                    if cur_g is not None:
                        cg, it = cur_g
                        try:
                            next(it)
                        except StopIteration:
                            emit_step(cg, 0)
                            tsteps[cg] = 1
                            ready.append(cg)
                            cur_g = None

            # ---- attention scores (per-window dir-sums, 12 matmuls) ----
            score_ps = ptg_pool.tile([128, NW], F32, tag="tgps")
            for wi in range(NW):
                for k in range(HC):
                    nc.tensor.matmul(
                        score_ps[:, wi:wi + 1],
                        lhsT=prsum[wi][:, k * TPC:(k + 1) * TPC],
                        rhs=ones[:],
                        start=(k == 0),
                        stop=(k == HC - 1),
                    )
            # softmax over the NW axis via tanh:
            # e^x = (1+tanh(x/2))/(1-tanh(x/2)) — avoids the exp
            # activation-table reload on the critical tail
            th = tmp_pool.tile([128, NW], F32, tag="th")
            nc.scalar.activation(th[:], score_ps[:], AF.Tanh,
                                 scale=float(0.5 * SCALE))
            den = tmp_pool.tile([128, NW], F32, tag="den")
            nc.vector.tensor_scalar(den[:], th[:], -1.0, 1.0, ALU.mult,
                                    ALU.add)
            dmx = tmp_pool.tile([128, NW], F32, tag="dmx")
            nc.vector.tensor_scalar(dmx[:], den[:], 1e-30, None, ALU.max)
            rden = tmp_pool.tile([128, NW], F32, tag="rden")
            nc.vector.reciprocal(rden[:], dmx[:])
            num = tmp_pool.tile([128, NW], F32, tag="num")
            nc.vector.tensor_scalar(num[:], th[:], 1.0, None, ALU.add)
            ex = tmp_pool.tile([128, NW], F32, tag="ex")
            nc.vector.tensor_tensor(ex[:], num[:], rden[:], ALU.mult)
            sm = tmp_pool.tile([128, 1], F32, tag="sm")
            nc.vector.reduce_sum(sm[:], ex[:], axis=AX.X)
            rs = tmp_pool.tile([128, 1], F32, tag="rs")
            nc.vector.reciprocal(rs[:], sm[:])
            attn = tmp_pool.tile([128, NW], BF16, tag="attn")
            nc.vector.tensor_scalar(attn[:], ex[:], rs[:], None, ALU.mult)
            # per-window: transpose attn column to [1,128], replicate to
            # [1,384], outer-product with ones to broadcast over partitions
            bcs = []
            for wi in range(NW):
                at_ps = ptg_pool.tile([1, TPC], BF16, tag="tgps",
                                      name=f"atps{wi}")
                nc.tensor.transpose(at_ps[:], attn[:, wi:wi + 1], idb[:])
                at_sb = tmp_pool.tile([1, HC * TPC], BF16, tag="atsb",
                                      name=f"atsb{wi}")
                for k in range(HC):
                    nc.vector.tensor_copy(at_sb[:, k * TPC:(k + 1) * TPC],
                                          at_ps[:])
                bc_ps = ptg_pool.tile([128, HC * TPC], F32, tag="tgps",
                                      name=f"bcps{wi}")
                nc.tensor.matmul(
                    bc_ps[:], lhsT=onesr[:, 0:128], rhs=at_sb[:],
                    start=True, stop=True,
                )
                bc = tmp_pool.tile([128, HC * TPC], BF16, tag="bc",
                                   name=f"bc{wi}", bufs=NW)
                nc.vector.tensor_copy(bc[:], bc_ps[:])
                bcs.append(bc)
            accs = []
            for d in (0, 1):
                eng = nc.vector if d == 0 else nc.gpsimd
                acc = fin_pool.tile([128, HC * TPC], BF16, tag=f"acc{d}")
                t1 = tmp_pool.tile([128, HC * TPC], BF16, tag=f"t1{d}")
                eng.tensor_tensor(t1[:], muts[d][:], bcs[0][:], ALU.mult)
                for wi in range(1, NW):
                    t2 = tmp_pool.tile([128, HC * TPC], BF16, tag=f"t2{d}")
                    eng.tensor_tensor(
                        t2[:], muts[wi * 2 + d][:], bcs[wi][:], ALU.mult)
                    eng.tensor_tensor(
                        t1[:] if wi < NW - 1 else acc[:], t1[:], t2[:],
                        ALU.add)
                # residual: out = x + local_feat
                eng.tensor_tensor(
                    acc[:], acc[:], xb3[:, d * HC:(d + 1) * HC, :], ALU.add)
                accs.append(acc)
            # ---- linear head: logits [9, 128] ----
            lg_ps = ptg_pool.tile([NL, TPC], F32, tag="tgps")
            for ci in range(DC):
                d = ci // HC
                k = ci % HC
                nc.tensor.matmul(
                    lg_ps[:],
                    lhsT=linw[:, ci * NL:(ci + 1) * NL],
                    rhs=accs[d][:, k * TPC:(k + 1) * TPC],
                    start=(ci == 0),
                    stop=False,
                )
            nc.tensor.matmul(
                lg_ps[:], lhsT=linb[:], rhs=onesr[:, 0:TPC],
                start=False, stop=True,
            )
            ob = fin_pool.tile([NL, TPC], F32, tag="ob")
            nc.vector.tensor_copy(ob[:], lg_ps[:])
            nc.sync.dma_start(out=out_d[:], in_=ob[:])

    nc.finalize()
    return nc


def _valid_scatter_np(x, valid_ids):
    Bx, Lx, Dx = x.shape
    v = (valid_ids == 1)
    out = np.zeros_like(x)
    for b in range(Bx):
        sel = x[b][v[b]]
        out[b, :sel.shape[0]] = sel
    return out


def _to_fp8(a):
    f8 = mybir.dt.np(FP8)  # ml_dtypes.float8_e4m3 (TRN flavor, max 240)
    return np.clip(np.asarray(a, np.float32), -240.0, 240.0).astype(f8)


def _sb_layout(a, nchunk):
    """[nchunk*128, N] -> [128, nchunk*N] in SBUF (p, k, n) layout."""
    n = a.shape[-1]
    return np.ascontiguousarray(
        a.reshape(nchunk, 128, n).transpose(1, 0, 2).reshape(128, nchunk * n))


def _host_prep(inputs):
    seq_out = np.asarray(inputs["seq_out"], np.float32)
    valid_ids = np.asarray(inputs["valid_ids"])
    x = _valid_scatter_np(seq_out, valid_ids)  # [B,L,D] f32

    bf = ml_dtypes.bfloat16
    f8 = mybir.dt.np(FP8)
    # gate permutation [i,f,g,o] -> [i,f,o,g]
    perm = np.concatenate([
        np.arange(0, H), np.arange(H, 2 * H),
        np.arange(3 * H, 4 * H), np.arange(2 * H, 3 * H),
    ])
    id8 = np.eye(128, dtype=f8)
    # weights, chain order c = window_idx*2 + dir (0=f, 1=b)
    wih = np.empty((NCH, 128, DC * G4), f8)
    whh = np.empty((NCH, 128, 2 * G4), f8)
    w2x = np.empty((NCH, 128, (GC + 1) * 128), f8)
    biasv = np.empty((128, NCH * GC), np.float32)
    for wi in range(NW):
        for d, sfx in ((0, "f"), (1, "b")):
            c = wi * 2 + d
            wih_c = _to_fp8(
                WS * np.asarray(inputs[f"w_ih_{sfx}"][wi], np.float32)[perm].T)
            whh_c = _to_fp8(
                WS * np.asarray(inputs[f"w_hh_{sfx}"][wi], np.float32)[perm].T)
            wih[c] = _sb_layout(wih_c, DC)
            whh[c] = _sb_layout(whh_c[0:256], 2)
            w2x[c] = np.concatenate([id8, whh_c[256:384]], axis=1)
            bv = (np.asarray(inputs[f"b_ih_{sfx}"][wi], np.float32)
                  + np.asarray(inputs[f"b_hh_{sfx}"][wi], np.float32))[perm]
            biasv[:, c * GC:(c + 1) * GC] = WS * bv.reshape(GC, 128).T
    linw = _sb_layout(
        np.asarray(inputs["lin_w"], np.float32).T.astype(bf), DC)  # [128,6*9]
    linb = np.asarray(inputs["lin_b"], np.float32)[None, :].astype(bf)
    idb = np.eye(128, dtype=bf)
    ones = np.ones((128, 1), bf)
    onesr = np.ones((1, 512), bf)

    in_maps = []
    for core in range(NCORES):
        b = core // 2
        right = core % 2  # 0: row-start half, 1: row-end half
        t0 = right * TPC
        # halo slice [t0-4, t0+132) of row b, zero-padded outside [0, L)
        xh = np.zeros((TH, D), np.float32)
        lo = max(0, t0 - HALO)
        hi = min(L, t0 + TPC + HALO)
        xh[lo - (t0 - HALO):hi - (t0 - HALO)] = x[b, lo:hi]
        xt = _sb_layout(_to_fp8(np.ascontiguousarray(xh.T)), DC)  # [128,6*136]
        xbc = _sb_layout(
            np.ascontiguousarray(x[b, t0:t0 + TPC].T).astype(bf), DC)
        # G-edge preds: invalid halo token columns (per chunk, 4 cols)
        pgl = np.full((128, GC * 5), 0 if right else 1, np.uint8)
        pgr = np.full((128, GC * 5), 1 if right else 0, np.uint8)
        biasc = np.empty((128, NCH * GC + 30), np.float32)
        biasc[:, :NCH * GC] = biasv
        biasc[:, NCH * GC:NCH * GC + 15] = pgl.view(np.float32)
        biasc[:, NCH * GC + 15:NCH * GC + 30] = pgr.view(np.float32)
        # muti merge preds: per window, 1s on the `half` trailing-edge slots
        pmf = np.zeros((128, NW * HC * 5), np.uint8)
        pmb = np.zeros((128, NW * HC * 5), np.uint8)
        for wi, w in enumerate(WINDOWS):
            half = w // 2
            for k in range(HC):
                base = (wi * HC + k) * 5
                if right:  # fwd trailing at row end: slots 4-half..3
                    pmf[:, base + 4 - half:base + 4] = 1
                else:      # bwd trailing at row start: slots 0..half-1
                    pmb[:, base:base + half] = 1
        in_maps.append({
            "xt": xt, "xb": xbc,
            "wih": wih, "whh": whh, "w2x": w2x, "bias": biasc,
            "linw": linw, "linb": linb,
            "idb": idb, "ones": ones, "onesr": onesr,
            "pmf": pmf, "pmb": pmb,
        })
    return in_maps


def kernel(**inputs) -> np.ndarray:
    if "nc" not in _CACHE:
        _CACHE["nc"] = _build()
    nc = _CACHE["nc"]
    in_maps = _host_prep(inputs)
    res = run_bass_kernel_spmd(nc, in_maps, core_ids=list(range(NCORES)))
    out = np.empty((B, L, NL), np.float32)
    for core in range(NCORES):
        b = core // 2
        t0 = (core % 2) * TPC
        out[b, t0:t0 + TPC] = res.results[core]["out"].T
    return out
